# revision 32
# baseline (speedup 1.0000x reference)
"""Channel-attention kernel for Trainium2 (8 NeuronCores).

Reference computation (per batch b):
    q = inputs[b].reshape(N, C)              # N = D*H*W = 4608, C = 64
    E = q @ q.T                              # (N, N)
    A = softmax(E, axis=-1)
    out[b] = gamma * (A @ q) + inputs[b]

Sharding: 8 cores = 4 batches x 2 row-halves of the attention matrix.
Each core computes softmax rows [r0, r0+2304) for one batch. Softmax is
row-wise, so no cross-core communication is needed.

Per-core algorithm (single pass over the attention matrix, transposed
layout, no on-chip transposes of the big matrix):
  * E'^T[m, n] = E[n, m] - sq[n] is computed directly by the QK^T matmul
    with an augmented contraction row: lhsT = [q^T; -1], rhs = [q^T; sq],
    where sq[n] = ||q_n||^2 stabilizes exp (the diagonal is ~the row max,
    and any per-row offset cancels in the softmax normalization).
  * U^T = exp(E'^T) on ScalarE straight out of PSUM (2 chunks per
    ACTIVATE to amortize the ~290-cycle fixed cost).
  * PV matmul with lhsT = [q | 1] computes both the unnormalized output
    (rows 0..63) and the softmax denominator S[n] (row 64) in one PSUM
    accumulation group.
  * Small PE transpose back to [n, c] layout, then out = U/S * gamma + x.
"""

import sys

for _p in ("/opt/trn_rl_repo",):
    if _p not in sys.path:
        sys.path.insert(0, _p)

import numpy as np
from contextlib import ExitStack

import concourse.bacc as bacc
import concourse.tile as tile
from concourse import mybir
from concourse import bass_utils

B, D, H, W_, C = 4, 8, 24, 24, 64
N = D * H * W_            # 4608
NCORES = 8
R = N // 2                # 2304 softmax rows per core
MCH = N // 128            # 36 contraction chunks
SUPERS = [512, 512, 512, 512, 256]   # n-column superblocks (sum = R)
DT = mybir.dt
AF = mybir.ActivationFunctionType

# QK_MODE: "f32r" = one float32r matmul with aug row (fast when PE warm,
#          ~fp32 accuracy); "bf16_split" = q split into bf16 hi+lo, two
#          bf16 matmuls (fp32-like accuracy); "bf16" = one bf16 matmul
#          (fastest, ~1% error).
QK_MODE = "f32r"
PV_DT = DT.float32r       # dtype of U tiles + q_aug for the PV matmul


def _build(qk_mode=None, pv_dt=None):
    qk_mode = qk_mode or QK_MODE
    pv_dt = pv_dt or PV_DT
    if qk_mode == "bf16_split":
        raise NotImplementedError("bf16_split dropped in asymmetric-psum layout")
    bf = DT.bfloat16
    nc = bacc.Bacc("TRN2", target_bir_lowering=False, debug=False)

    if qk_mode == "f32r":
        lhs_a = nc.dram_tensor("lhs_a", (65, N), DT.float32, kind="ExternalInput").ap()
        rhs_a = nc.dram_tensor("rhs_a", (65, R), DT.float32, kind="ExternalInput").ap()
    elif qk_mode == "bf16_split":
        lhs_hl = nc.dram_tensor("lhs_hl", (128, N), bf, kind="ExternalInput").ap()
        rhs_hh = nc.dram_tensor("rhs_hh", (128, R), bf, kind="ExternalInput").ap()
        lhs_a = nc.dram_tensor("lhs_a", (65, N), bf, kind="ExternalInput").ap()
        rhs_a = nc.dram_tensor("rhs_a", (65, R), bf, kind="ExternalInput").ap()
    elif qk_mode == "bf16":
        lhs_a = nc.dram_tensor("lhs_a", (65, N), bf, kind="ExternalInput").ap()
        rhs_a = nc.dram_tensor("rhs_a", (65, R), bf, kind="ExternalInput").ap()
    else:
        raise ValueError(qk_mode)

    q_aug = nc.dram_tensor("q_aug", (N, 65), DT.float32, kind="ExternalInput").ap()
    x_res = nc.dram_tensor("x_res", (R, C), DT.float32, kind="ExternalInput").ap()
    gvec = nc.dram_tensor("gvec", (128, 1), DT.float32, kind="ExternalInput").ap()
    ident = nc.dram_tensor("ident", (128, 128), DT.float32, kind="ExternalInput").ap()
    out = nc.dram_tensor("out", (R, C), DT.float32, kind="ExternalOutput").ap()

    qk_dt = DT.float32r if qk_mode == "f32r" else bf

    with ExitStack() as ctx:
        tc = ctx.enter_context(tile.TileContext(nc))
        singles = ctx.enter_context(tc.tile_pool(name="singles", bufs=1))
        upool = ctx.enter_context(tc.tile_pool(name="u", bufs=4))
        opool = ctx.enter_context(tc.tile_pool(name="o", bufs=3))
        spool = ctx.enter_context(tc.tile_pool(name="s", bufs=4))
        # PSUM budget (8 banks): pe 3x2 + po 1 + pt 1 — the 3-deep pe pool
        # lets the PE run up to 3 chunk-pairs ahead of ScalarE, which keeps
        # both engines packed. Measured better than pe 2x2/po 2/pt 2 (122us
        # -> 109us) and than a single-buffered 4-bank/2048-wide-EXP layout
        # (131us: the wider EXP saves ~7us of ACT work but loses the
        # lookahead depth, leaving ScalarE only ~70% occupied).
        pe_pool = ctx.enter_context(tc.tile_pool(name="pe", bufs=3, space="PSUM"))
        po_pool = ctx.enter_context(tc.tile_pool(name="po", bufs=2, space="PSUM"))
        pt_pool = pe_pool  # transposes borrow pe slots at super boundaries

        # Stage input DMAs in first-use order across two rings (sync HWDGE
        # for the QK operands, gpsimd SWDGE for the PV/epilogue operands)
        # so the main loop can start ~4us in instead of waiting ~15us for
        # everything.
        la_src = lhs_a.bitcast(qk_dt) if qk_mode == "f32r" else lhs_a
        ra_src = rhs_a.bitcast(qk_dt) if qk_mode == "f32r" else rhs_a
        LA = singles.tile([65, N], qk_dt)
        RA = singles.tile([65, R], qk_dt)
        nc.sync.dma_start(out=RA[:, :512], in_=ra_src[:, :512])
        for a in range(0, MCH, 9):
            sl = slice(a * 128, (a + 9) * 128)
            nc.sync.dma_start(out=LA[:, sl], in_=la_src[:, sl])
        nc.sync.dma_start(out=RA[:, 512:], in_=ra_src[:, 512:])
        if qk_mode == "bf16_split":
            LHL = singles.tile([128, N], bf)
            nc.sync.dma_start(out=LHL, in_=lhs_hl)
            RHH = singles.tile([128, R], bf)
            nc.sync.dma_start(out=RHH, in_=rhs_hh)

        qa_src = (
            q_aug.bitcast(DT.float32r) if pv_dt == DT.float32r else q_aug
        ).rearrange("(t p) c -> p t c", p=128)
        QA = singles.tile([128, MCH, 65], pv_dt)
        for a in range(0, MCH, 6):
            nc.gpsimd.dma_start(out=QA[:, a : a + 6, :], in_=qa_src[:, a : a + 6, :])
        ID = singles.tile([128, 128], DT.float32)
        nc.gpsimd.dma_start(out=ID, in_=ident)
        G = singles.tile([128, 1], DT.float32)
        nc.gpsimd.dma_start(out=G, in_=gvec)
        XR = singles.tile([128, R // 128, C], DT.float32)
        nc.gpsimd.dma_start(out=XR, in_=x_res.rearrange("(t p) c -> p t c", p=128))

        out_r = out.rearrange("(t p) c -> t p c", p=128)

        # HAM warmup: fp32-mode matmuls don't un-throttle the PE clock
        # (observed: all-fp32r kernel runs at 1.2 GHz forever). Issue ~5us
        # of bf16 dummy matmuls first — they overlap the input DMAs, warm
        # the PE to 2.4 GHz, and the fp32r stream then retains warmth.
        wz = singles.tile([128, 512], DT.bfloat16)
        nc.vector.memset(wz, 0.0)
        # Pre-load the Exp spline table off the critical path (overlaps the
        # input DMAs) so group 0's first EXP doesn't stall ~2.7us.
        sc0 = spool.tile([128, 1], DT.float32, tag="sc0")
        nc.vector.memset(sc0, 0.0)
        tl0 = spool.tile([128, 1], DT.float32, tag="tl0")
        nc.scalar.activation(tl0, sc0, AF.Exp)
        for w_ in range(4):
            ew = pe_pool.tile([128, 2, 512], DT.float32, tag="pe")
            for k_ in range(2):
                nc.tensor.matmul(ew[:, k_, :], wz[:, :128], wz, start=True, stop=True)

        col = 0
        for s, Wd in enumerate(SUPERS):
            po_t = po_pool.tile([128, 512], DT.float32, tag="po")
            nsl = slice(col, col + Wd)
            per_bank = 512 // Wd
            n_pack = 2 * per_bank     # chunks per [128, 2, 512] tile
            for t in range(0, MCH, n_pack):
                e = pe_pool.tile([128, 2, 512], DT.float32, tag="pe")
                u = upool.tile([128, 2, 512], pv_dt, tag="u")
                for d_ in range(n_pack):
                    tc_ = t + d_
                    ch = slice(tc_ * 128, (tc_ + 1) * 128)
                    nc.tensor.matmul(
                        e[:, d_ // per_bank, (d_ % per_bank) * Wd :][:, :Wd],
                        LA[:, ch],
                        RA[:, nsl],
                        start=True,
                        stop=True,
                    )
                nc.scalar.activation(u, e, AF.Exp)
                for d_ in range(n_pack):
                    tc_ = t + d_
                    nc.tensor.matmul(
                        po_t[:65, :Wd],
                        QA[:, tc_, :],
                        u[:, d_ // per_bank, (d_ % per_bank) * Wd :][:, :Wd],
                        start=(tc_ == 0),
                        stop=(tc_ == MCH - 1),
                    )
            # po_t rows 0..63 = unnormalized out^T, row 64 = S[n]
            ps = opool.tile([65, 512], DT.float32, tag="ps")
            nc.vector.tensor_copy(ps[:, :Wd], po_t[:65, :Wd])
            for j in range(Wd // 128):
                tp = pt_pool.tile([128, 65], DT.float32, tag="pe")
                nc.tensor.transpose(tp, ps[:, j * 128 : (j + 1) * 128], ID[:65, :65])
                rs = spool.tile([128, 1], DT.float32, tag="rs")
                nc.vector.reciprocal(rs, tp[:, 64:65])
                nc.vector.tensor_mul(rs, rs, G)
                of = opool.tile([128, C], DT.float32, tag="of")
                nc.vector.tensor_scalar_mul(of, tp[:, 0:64], rs)
                blk = col // 128 + j
                nc.vector.tensor_add(of, of, XR[:, blk, :])
                nc.sync.dma_start(out=out_r[blk], in_=of)
            col += Wd

    nc.compile()
    return nc


_CACHE = {}


def get_nc():
    key = (QK_MODE, PV_DT)
    if key not in _CACHE:
        _CACHE[key] = _build()
    return _CACHE[key]


def _bf16(a):
    """Round-to-nearest-even float32 -> bfloat16 (as uint16 bit pattern)."""
    u = a.view(np.uint32)
    rounded = ((u + 0x7FFF + ((u >> 16) & 1)) >> 16).astype(np.uint16)
    return rounded


def _bf16_to_f32(b):
    return (b.astype(np.uint32) << 16).view(np.float32)


def make_in_maps(inputs_arr, gamma):
    q_all = np.ascontiguousarray(
        np.asarray(inputs_arr, dtype=np.float32).reshape(B, N, C)
    )
    gv = np.full((128, 1), np.float32(gamma), dtype=np.float32)
    ident = np.eye(128, dtype=np.float32)
    in_maps = []
    for core in range(NCORES):
        b, h = core // 2, core % 2
        qb = q_all[b]                               # (N, C)
        qbT = np.ascontiguousarray(qb.T)            # (C, N)
        sq = np.einsum("nc,nc->n", qb, qb).astype(np.float32)
        r0 = h * R
        m = dict(gvec=gv, ident=ident, x_res=np.ascontiguousarray(qb[r0 : r0 + R]))

        q_aug = np.empty((N, 65), np.float32)
        q_aug[:, :64] = qb
        q_aug[:, 64] = 1.0
        m["q_aug"] = q_aug

        if QK_MODE == "f32r":
            lhs_a = np.empty((65, N), np.float32)
            lhs_a[:64] = qbT
            lhs_a[64] = -1.0
            rhs_a = np.empty((65, R), np.float32)
            rhs_a[:64] = qbT[:, r0 : r0 + R]
            rhs_a[64] = sq[r0 : r0 + R]
            m["lhs_a"], m["rhs_a"] = lhs_a, rhs_a
        else:
            hiT = _bf16(qbT)                        # (64, N) uint16 bf16 bits
            if QK_MODE == "bf16_split":
                loT = _bf16(qbT - _bf16_to_f32(hiT))
                lhs_hl = np.concatenate([hiT, loT], axis=0)       # (128, N)
                rhs_hh = np.concatenate(
                    [hiT[:, r0 : r0 + R], hiT[:, r0 : r0 + R]], axis=0
                )
                lhs_a = np.concatenate(
                    [hiT, _bf16(np.full((1, N), -1.0, np.float32))], axis=0
                )
                rhs_a = np.concatenate(
                    [loT[:, r0 : r0 + R], _bf16(sq[None, r0 : r0 + R])], axis=0
                )
                m["lhs_hl"], m["rhs_hh"] = lhs_hl, rhs_hh
                m["lhs_a"], m["rhs_a"] = lhs_a, rhs_a
            else:  # plain bf16
                lhs_a = np.concatenate(
                    [hiT, _bf16(np.full((1, N), -1.0, np.float32))], axis=0
                )
                rhs_a = np.concatenate(
                    [hiT[:, r0 : r0 + R], _bf16(sq[None, r0 : r0 + R])], axis=0
                )
                m["lhs_a"], m["rhs_a"] = lhs_a, rhs_a
        in_maps.append(m)
    return in_maps


def run_hw(in_maps, **kwargs):
    nc = get_nc()
    return bass_utils.run_bass_kernel_spmd(
        nc, in_maps, core_ids=list(range(NCORES)), **kwargs
    )


def assemble(results):
    out_full = np.empty((B, N, C), np.float32)
    for core in range(NCORES):
        b, h = core // 2, core % 2
        out_full[b, h * R : (h + 1) * R] = results[core]["out"]
    return out_full.reshape(B, D, H, W_, C)


def kernel(**inputs):
    inputs_arr = np.asarray(inputs["inputs"], dtype=np.float32)
    gamma = np.asarray(inputs["gamma"], dtype=np.float32).reshape(-1)[0]
    in_maps = make_in_maps(inputs_arr, gamma)
    try:
        res = run_hw(in_maps)
    except Exception:
        import time

        time.sleep(5)
        res = run_hw(in_maps)
    return assemble(res.results)


# revision 33
# speedup vs baseline: 1.1798x; 1.1798x over previous
"""Channel-attention kernel for Trainium2 (8 NeuronCores).

Reference computation (per batch b):
    q = inputs[b].reshape(N, C)              # N = D*H*W = 4608, C = 64
    E = q @ q.T                              # (N, N)
    A = softmax(E, axis=-1)
    out[b] = gamma * (A @ q) + inputs[b]

Sharding: 8 cores = 4 batches x 2 row-halves of the attention matrix.
Each core computes softmax rows [r0, r0+2304) for one batch. Softmax is
row-wise, so no cross-core communication is needed.

Per-core algorithm (single pass over the attention matrix, transposed
layout, no on-chip transposes of the big matrix):
  * E'^T[m, n] = E[n, m] - sq[n] is computed directly by the QK^T matmul
    with an augmented contraction row: lhsT = [q^T; -1], rhs = [q^T; sq],
    where sq[n] = ||q_n||^2 stabilizes exp (the diagonal is ~the row max,
    and any per-row offset cancels in the softmax normalization).
  * U^T = exp(E'^T) on ScalarE straight out of PSUM (2 chunks per
    ACTIVATE to amortize the ~290-cycle fixed cost).
  * PV matmul with lhsT = [q | 1] computes both the unnormalized output
    (rows 0..63) and the softmax denominator S[n] (row 64) in one PSUM
    accumulation group.
  * Small PE transpose back to [n, c] layout, then out = U/S * gamma + x.
"""

import sys

for _p in ("/opt/trn_rl_repo",):
    if _p not in sys.path:
        sys.path.insert(0, _p)

import numpy as np
from contextlib import ExitStack

import concourse.bacc as bacc
import concourse.tile as tile
from concourse import mybir
from concourse import bass_utils

B, D, H, W_, C = 4, 8, 24, 24, 64
N = D * H * W_            # 4608
NCORES = 8
R = N // 2                # 2304 softmax rows per core
MCH = N // 128            # 36 contraction chunks
SUPERS = [512, 512, 512, 512, 256]   # n-column superblocks (sum = R)
DT = mybir.dt
AF = mybir.ActivationFunctionType

# QK_MODE: "f32r" = one float32r matmul with aug row (fast when PE warm,
#          ~fp32 accuracy); "bf16_split" = q split into bf16 hi+lo, two
#          bf16 matmuls (fp32-like accuracy); "bf16" = one bf16 matmul
#          (fastest, ~1% error).
QK_MODE = "f32r"
PV_DT = DT.float32r       # dtype of U tiles + q_aug for the PV matmul


def _build(qk_mode=None, pv_dt=None):
    qk_mode = qk_mode or QK_MODE
    pv_dt = pv_dt or PV_DT
    if qk_mode == "bf16_split":
        raise NotImplementedError("bf16_split dropped in asymmetric-psum layout")
    bf = DT.bfloat16
    nc = bacc.Bacc("TRN2", target_bir_lowering=False, debug=False)

    if qk_mode == "f32r":
        lhs_a = nc.dram_tensor("lhs_a", (65, N), DT.float32, kind="ExternalInput").ap()
        rhs_a = nc.dram_tensor("rhs_a", (65, R), DT.float32, kind="ExternalInput").ap()
    elif qk_mode == "bf16_split":
        lhs_hl = nc.dram_tensor("lhs_hl", (128, N), bf, kind="ExternalInput").ap()
        rhs_hh = nc.dram_tensor("rhs_hh", (128, R), bf, kind="ExternalInput").ap()
        lhs_a = nc.dram_tensor("lhs_a", (65, N), bf, kind="ExternalInput").ap()
        rhs_a = nc.dram_tensor("rhs_a", (65, R), bf, kind="ExternalInput").ap()
    elif qk_mode == "bf16":
        lhs_a = nc.dram_tensor("lhs_a", (65, N), bf, kind="ExternalInput").ap()
        rhs_a = nc.dram_tensor("rhs_a", (65, R), bf, kind="ExternalInput").ap()
    else:
        raise ValueError(qk_mode)

    q_aug = nc.dram_tensor("q_aug", (N, 65), DT.float32, kind="ExternalInput").ap()
    x_res = nc.dram_tensor("x_res", (R, C), DT.float32, kind="ExternalInput").ap()
    gvec = nc.dram_tensor("gvec", (128, 1), DT.float32, kind="ExternalInput").ap()
    ident = nc.dram_tensor("ident", (128, 128), DT.float32, kind="ExternalInput").ap()
    out = nc.dram_tensor("out", (R, C), DT.float32, kind="ExternalOutput").ap()

    qk_dt = DT.float32r if qk_mode == "f32r" else bf

    with ExitStack() as ctx:
        tc = ctx.enter_context(tile.TileContext(nc))
        singles = ctx.enter_context(tc.tile_pool(name="singles", bufs=1))
        upool = ctx.enter_context(tc.tile_pool(name="u", bufs=4))
        opool = ctx.enter_context(tc.tile_pool(name="o", bufs=3))
        spool = ctx.enter_context(tc.tile_pool(name="s", bufs=4))
        # PSUM budget (8 banks): pe 3x2 + po 1 + pt 1 — the 3-deep pe pool
        # lets the PE run up to 3 chunk-pairs ahead of ScalarE, which keeps
        # both engines packed. Measured better than pe 2x2/po 2/pt 2 (122us
        # -> 109us) and than a single-buffered 4-bank/2048-wide-EXP layout
        # (131us: the wider EXP saves ~7us of ACT work but loses the
        # lookahead depth, leaving ScalarE only ~70% occupied).
        pe_pool = ctx.enter_context(tc.tile_pool(name="pe", bufs=3, space="PSUM"))
        po_pool = ctx.enter_context(tc.tile_pool(name="po", bufs=1, space="PSUM"))
        pt_pool = ctx.enter_context(tc.tile_pool(name="pt", bufs=1, space="PSUM"))

        # Stage input DMAs in first-use order across two rings (sync HWDGE
        # for the QK operands, gpsimd SWDGE for the PV/epilogue operands)
        # so the main loop can start ~4us in instead of waiting ~15us for
        # everything.
        la_src = lhs_a.bitcast(qk_dt) if qk_mode == "f32r" else lhs_a
        ra_src = rhs_a.bitcast(qk_dt) if qk_mode == "f32r" else rhs_a
        LA = singles.tile([65, N], qk_dt)
        RA = singles.tile([65, R], qk_dt)
        nc.sync.dma_start(out=RA[:, :512], in_=ra_src[:, :512])
        for a in range(0, MCH, 9):
            sl = slice(a * 128, (a + 9) * 128)
            nc.sync.dma_start(out=LA[:, sl], in_=la_src[:, sl])
        nc.sync.dma_start(out=RA[:, 512:], in_=ra_src[:, 512:])
        if qk_mode == "bf16_split":
            LHL = singles.tile([128, N], bf)
            nc.sync.dma_start(out=LHL, in_=lhs_hl)
            RHH = singles.tile([128, R], bf)
            nc.sync.dma_start(out=RHH, in_=rhs_hh)

        qa_src = (
            q_aug.bitcast(DT.float32r) if pv_dt == DT.float32r else q_aug
        ).rearrange("(t p) c -> p t c", p=128)
        QA = singles.tile([128, MCH, 65], pv_dt)
        for a in range(0, MCH, 6):
            nc.gpsimd.dma_start(out=QA[:, a : a + 6, :], in_=qa_src[:, a : a + 6, :])
        ID = singles.tile([128, 128], DT.float32)
        nc.gpsimd.dma_start(out=ID, in_=ident)
        G = singles.tile([128, 1], DT.float32)
        nc.gpsimd.dma_start(out=G, in_=gvec)
        XR = singles.tile([128, R // 128, C], DT.float32)
        nc.gpsimd.dma_start(out=XR, in_=x_res.rearrange("(t p) c -> p t c", p=128))

        out_r = out.rearrange("(t p) c -> t p c", p=128)

        # HAM warmup: fp32-mode matmuls don't un-throttle the PE clock
        # (observed: all-fp32r kernel runs at 1.2 GHz forever). Issue ~5us
        # of bf16 dummy matmuls first — they overlap the input DMAs, warm
        # the PE to 2.4 GHz, and the fp32r stream then retains warmth.
        wz = singles.tile([128, 512], DT.bfloat16)
        nc.vector.memset(wz, 0.0)
        # Pre-load the Exp spline table off the critical path (overlaps the
        # input DMAs) so group 0's first EXP doesn't stall ~2.7us.
        sc0 = spool.tile([128, 1], DT.float32, tag="sc0")
        nc.vector.memset(sc0, 0.0)
        tl0 = spool.tile([128, 1], DT.float32, tag="tl0")
        nc.scalar.activation(tl0, sc0, AF.Exp)
        for w_ in range(4):
            ew = pe_pool.tile([128, 2, 512], DT.float32, tag="pe")
            for k_ in range(2):
                nc.tensor.matmul(ew[:, k_, :], wz[:, :128], wz, start=True, stop=True)

        col = 0
        for s, Wd in enumerate(SUPERS):
            po_t = po_pool.tile([128, 512], DT.float32, tag="po")
            nsl = slice(col, col + Wd)
            per_bank = 512 // Wd
            n_pack = 2 * per_bank     # chunks per [128, 2, 512] tile
            for t in range(0, MCH, n_pack):
                e = pe_pool.tile([128, 2, 512], DT.float32, tag="pe")
                u = upool.tile([128, 2, 512], pv_dt, tag="u")
                for d_ in range(n_pack):
                    tc_ = t + d_
                    ch = slice(tc_ * 128, (tc_ + 1) * 128)
                    nc.tensor.matmul(
                        e[:, d_ // per_bank, (d_ % per_bank) * Wd :][:, :Wd],
                        LA[:, ch],
                        RA[:, nsl],
                        start=True,
                        stop=True,
                    )
                nc.scalar.activation(u, e, AF.Exp)
                for d_ in range(n_pack):
                    tc_ = t + d_
                    nc.tensor.matmul(
                        po_t[:65, :Wd],
                        QA[:, tc_, :],
                        u[:, d_ // per_bank, (d_ % per_bank) * Wd :][:, :Wd],
                        start=(tc_ == 0),
                        stop=(tc_ == MCH - 1),
                    )
            # po_t rows 0..63 = unnormalized out^T, row 64 = S[n]
            ps = opool.tile([65, 512], DT.float32, tag="ps")
            nc.vector.tensor_copy(ps[:, :Wd], po_t[:65, :Wd])
            for j in range(Wd // 128):
                tp = pt_pool.tile([128, 65], DT.float32, tag="tp")
                nc.tensor.transpose(tp, ps[:, j * 128 : (j + 1) * 128], ID[:65, :65])
                rs = spool.tile([128, 1], DT.float32, tag="rs")
                nc.vector.reciprocal(rs, tp[:, 64:65])
                nc.vector.tensor_mul(rs, rs, G)
                of = opool.tile([128, C], DT.float32, tag="of")
                nc.vector.tensor_scalar_mul(of, tp[:, 0:64], rs)
                blk = col // 128 + j
                nc.vector.tensor_add(of, of, XR[:, blk, :])
                nc.sync.dma_start(out=out_r[blk], in_=of)
            col += Wd

    nc.compile()
    return nc


_CACHE = {}


def get_nc():
    key = (QK_MODE, PV_DT)
    if key not in _CACHE:
        _CACHE[key] = _build()
    return _CACHE[key]


def _bf16(a):
    """Round-to-nearest-even float32 -> bfloat16 (as uint16 bit pattern)."""
    u = a.view(np.uint32)
    rounded = ((u + 0x7FFF + ((u >> 16) & 1)) >> 16).astype(np.uint16)
    return rounded


def _bf16_to_f32(b):
    return (b.astype(np.uint32) << 16).view(np.float32)


def make_in_maps(inputs_arr, gamma):
    q_all = np.ascontiguousarray(
        np.asarray(inputs_arr, dtype=np.float32).reshape(B, N, C)
    )
    gv = np.full((128, 1), np.float32(gamma), dtype=np.float32)
    ident = np.eye(128, dtype=np.float32)
    in_maps = []
    for core in range(NCORES):
        b, h = core // 2, core % 2
        qb = q_all[b]                               # (N, C)
        qbT = np.ascontiguousarray(qb.T)            # (C, N)
        sq = np.einsum("nc,nc->n", qb, qb).astype(np.float32)
        r0 = h * R
        m = dict(gvec=gv, ident=ident, x_res=np.ascontiguousarray(qb[r0 : r0 + R]))

        q_aug = np.empty((N, 65), np.float32)
        q_aug[:, :64] = qb
        q_aug[:, 64] = 1.0
        m["q_aug"] = q_aug

        if QK_MODE == "f32r":
            lhs_a = np.empty((65, N), np.float32)
            lhs_a[:64] = qbT
            lhs_a[64] = -1.0
            rhs_a = np.empty((65, R), np.float32)
            rhs_a[:64] = qbT[:, r0 : r0 + R]
            rhs_a[64] = sq[r0 : r0 + R]
            m["lhs_a"], m["rhs_a"] = lhs_a, rhs_a
        else:
            hiT = _bf16(qbT)                        # (64, N) uint16 bf16 bits
            if QK_MODE == "bf16_split":
                loT = _bf16(qbT - _bf16_to_f32(hiT))
                lhs_hl = np.concatenate([hiT, loT], axis=0)       # (128, N)
                rhs_hh = np.concatenate(
                    [hiT[:, r0 : r0 + R], hiT[:, r0 : r0 + R]], axis=0
                )
                lhs_a = np.concatenate(
                    [hiT, _bf16(np.full((1, N), -1.0, np.float32))], axis=0
                )
                rhs_a = np.concatenate(
                    [loT[:, r0 : r0 + R], _bf16(sq[None, r0 : r0 + R])], axis=0
                )
                m["lhs_hl"], m["rhs_hh"] = lhs_hl, rhs_hh
                m["lhs_a"], m["rhs_a"] = lhs_a, rhs_a
            else:  # plain bf16
                lhs_a = np.concatenate(
                    [hiT, _bf16(np.full((1, N), -1.0, np.float32))], axis=0
                )
                rhs_a = np.concatenate(
                    [hiT[:, r0 : r0 + R], _bf16(sq[None, r0 : r0 + R])], axis=0
                )
                m["lhs_a"], m["rhs_a"] = lhs_a, rhs_a
        in_maps.append(m)
    return in_maps


def run_hw(in_maps, **kwargs):
    nc = get_nc()
    return bass_utils.run_bass_kernel_spmd(
        nc, in_maps, core_ids=list(range(NCORES)), **kwargs
    )


def assemble(results):
    out_full = np.empty((B, N, C), np.float32)
    for core in range(NCORES):
        b, h = core // 2, core % 2
        out_full[b, h * R : (h + 1) * R] = results[core]["out"]
    return out_full.reshape(B, D, H, W_, C)


def kernel(**inputs):
    inputs_arr = np.asarray(inputs["inputs"], dtype=np.float32)
    gamma = np.asarray(inputs["gamma"], dtype=np.float32).reshape(-1)[0]
    in_maps = make_in_maps(inputs_arr, gamma)
    try:
        res = run_hw(in_maps)
    except Exception:
        import time

        time.sleep(5)
        res = run_hw(in_maps)
    return assemble(res.results)


# revision 35
# speedup vs baseline: 1.2367x; 1.0483x over previous
"""Channel-attention kernel for Trainium2 (8 NeuronCores).

Reference computation (per batch b):
    q = inputs[b].reshape(N, C)              # N = D*H*W = 4608, C = 64
    E = q @ q.T                              # (N, N)
    A = softmax(E, axis=-1)
    out[b] = gamma * (A @ q) + inputs[b]

Sharding: 8 cores = 4 batches x 2 row-halves of the attention matrix.
Each core computes softmax rows [r0, r0+2304) for one batch. Softmax is
row-wise, so no cross-core communication is needed.

Per-core algorithm (single pass over the attention matrix, transposed
layout, no on-chip transposes of the big matrix):
  * E'^T[m, n] = E[n, m] - sq[n] is computed directly by the QK^T matmul
    with an augmented contraction row: lhsT = [q^T; -1], rhs = [q^T; sq],
    where sq[n] = ||q_n||^2 stabilizes exp (the diagonal is ~the row max,
    and any per-row offset cancels in the softmax normalization).
  * U^T = exp(E'^T) on ScalarE straight out of PSUM (2 chunks per
    ACTIVATE to amortize the ~290-cycle fixed cost).
  * PV matmul with lhsT = [q | 1] computes both the unnormalized output
    (rows 0..63) and the softmax denominator S[n] (row 64) in one PSUM
    accumulation group.
  * Small PE transpose back to [n, c] layout, then out = U/S * gamma + x.
"""

import sys

for _p in ("/opt/trn_rl_repo",):
    if _p not in sys.path:
        sys.path.insert(0, _p)

import numpy as np
from contextlib import ExitStack

import concourse.bacc as bacc
import concourse.tile as tile
from concourse import mybir
from concourse import bass_utils

B, D, H, W_, C = 4, 8, 24, 24, 64
N = D * H * W_            # 4608
NCORES = 8
R = N // 2                # 2304 softmax rows per core
MCH = N // 128            # 36 contraction chunks
SUPERS = [512, 512, 512, 512, 256]   # n-column superblocks (sum = R)
DT = mybir.dt
AF = mybir.ActivationFunctionType

# QK_MODE: "f32r" = one float32r matmul with aug row (fast when PE warm,
#          ~fp32 accuracy); "bf16_split" = q split into bf16 hi+lo, two
#          bf16 matmuls (fp32-like accuracy); "bf16" = one bf16 matmul
#          (fastest, ~1% error).
QK_MODE = "f32r"
PV_DT = DT.float32r       # dtype of U tiles + q_aug for the PV matmul


def _build(qk_mode=None, pv_dt=None):
    qk_mode = qk_mode or QK_MODE
    pv_dt = pv_dt or PV_DT
    if qk_mode == "bf16_split":
        raise NotImplementedError("bf16_split dropped in asymmetric-psum layout")
    bf = DT.bfloat16
    nc = bacc.Bacc("TRN2", target_bir_lowering=False, debug=False)

    if qk_mode == "f32r":
        lhs_a = nc.dram_tensor("lhs_a", (65, N), DT.float32, kind="ExternalInput").ap()
        rhs_a = nc.dram_tensor("rhs_a", (65, R), DT.float32, kind="ExternalInput").ap()
    elif qk_mode == "bf16_split":
        lhs_hl = nc.dram_tensor("lhs_hl", (128, N), bf, kind="ExternalInput").ap()
        rhs_hh = nc.dram_tensor("rhs_hh", (128, R), bf, kind="ExternalInput").ap()
        lhs_a = nc.dram_tensor("lhs_a", (65, N), bf, kind="ExternalInput").ap()
        rhs_a = nc.dram_tensor("rhs_a", (65, R), bf, kind="ExternalInput").ap()
    elif qk_mode == "bf16":
        lhs_a = nc.dram_tensor("lhs_a", (65, N), bf, kind="ExternalInput").ap()
        rhs_a = nc.dram_tensor("rhs_a", (65, R), bf, kind="ExternalInput").ap()
    else:
        raise ValueError(qk_mode)

    q_aug = nc.dram_tensor("q_aug", (N, 65), DT.float32, kind="ExternalInput").ap()
    x_res = nc.dram_tensor("x_res", (R, C), DT.float32, kind="ExternalInput").ap()
    gvec = nc.dram_tensor("gvec", (128, 1), DT.float32, kind="ExternalInput").ap()
    ident = nc.dram_tensor("ident", (128, 128), DT.float32, kind="ExternalInput").ap()
    out = nc.dram_tensor("out", (R, C), DT.float32, kind="ExternalOutput").ap()

    qk_dt = DT.float32r if qk_mode == "f32r" else bf

    with ExitStack() as ctx:
        tc = ctx.enter_context(tile.TileContext(nc))
        singles = ctx.enter_context(tc.tile_pool(name="singles", bufs=1))
        upool = ctx.enter_context(tc.tile_pool(name="u", bufs=4))
        opool = ctx.enter_context(tc.tile_pool(name="o", bufs=3))
        spool = ctx.enter_context(tc.tile_pool(name="s", bufs=4))
        # PSUM budget (8 banks): pe 3x2 + po 1 + pt 1 — the 3-deep pe pool
        # lets the PE run up to 3 chunk-pairs ahead of ScalarE, which keeps
        # both engines packed. Measured better than pe 2x2/po 2/pt 2 (122us
        # -> 109us) and than a single-buffered 4-bank/2048-wide-EXP layout
        # (131us: the wider EXP saves ~7us of ACT work but loses the
        # lookahead depth, leaving ScalarE only ~70% occupied).
        pe_pool = ctx.enter_context(tc.tile_pool(name="pe", bufs=3, space="PSUM"))
        po_pool = ctx.enter_context(tc.tile_pool(name="po", bufs=1, space="PSUM"))
        pt_pool = ctx.enter_context(tc.tile_pool(name="pt", bufs=1, space="PSUM"))

        # Stage input DMAs in first-use order across two rings (sync HWDGE
        # for the QK operands, gpsimd SWDGE for the PV/epilogue operands)
        # so the main loop can start ~4us in instead of waiting ~15us for
        # everything.
        la_src = lhs_a.bitcast(qk_dt) if qk_mode == "f32r" else lhs_a
        ra_src = rhs_a.bitcast(qk_dt) if qk_mode == "f32r" else rhs_a
        LA = singles.tile([65, N], qk_dt)
        RA = singles.tile([65, R], qk_dt)
        nc.sync.dma_start(out=RA[:, :512], in_=ra_src[:, :512])
        for a in range(0, MCH, 9):
            sl = slice(a * 128, (a + 9) * 128)
            nc.sync.dma_start(out=LA[:, sl], in_=la_src[:, sl])
        nc.sync.dma_start(out=RA[:, 512:], in_=ra_src[:, 512:])
        if qk_mode == "bf16_split":
            LHL = singles.tile([128, N], bf)
            nc.sync.dma_start(out=LHL, in_=lhs_hl)
            RHH = singles.tile([128, R], bf)
            nc.sync.dma_start(out=RHH, in_=rhs_hh)

        qa_src = (
            q_aug.bitcast(DT.float32r) if pv_dt == DT.float32r else q_aug
        ).rearrange("(t p) c -> p t c", p=128)
        QA = singles.tile([128, MCH, 65], pv_dt)
        for a in range(0, MCH, 6):
            nc.gpsimd.dma_start(out=QA[:, a : a + 6, :], in_=qa_src[:, a : a + 6, :])
        ID = singles.tile([128, 128], DT.float32)
        nc.gpsimd.dma_start(out=ID, in_=ident)
        G = singles.tile([128, 1], DT.float32)
        nc.gpsimd.dma_start(out=G, in_=gvec)
        XR = singles.tile([128, R // 128, C], DT.float32)
        nc.gpsimd.dma_start(out=XR, in_=x_res.rearrange("(t p) c -> p t c", p=128))

        out_r = out.rearrange("(t p) c -> t p c", p=128)

        # HAM warmup: fp32-mode matmuls don't un-throttle the PE clock
        # (observed: all-fp32r kernel runs at 1.2 GHz forever). Issue ~5us
        # of bf16 dummy matmuls first — they overlap the input DMAs, warm
        # the PE to 2.4 GHz, and the fp32r stream then retains warmth.
        wz = singles.tile([128, 512], DT.bfloat16)
        nc.vector.memset(wz, 0.0)
        # Pre-load the Exp spline table off the critical path (overlaps the
        # input DMAs) so group 0's first EXP doesn't stall ~2.7us.
        sc0 = spool.tile([128, 1], DT.float32, tag="sc0")
        nc.vector.memset(sc0, 0.0)
        tl0 = spool.tile([128, 1], DT.float32, tag="tl0")
        nc.scalar.activation(tl0, sc0, AF.Exp)
        for w_ in range(4):
            ew = pe_pool.tile([128, 2, 512], DT.float32, tag="pe")
            for k_ in range(2):
                nc.tensor.matmul(ew[:, k_, :], wz[:, :128], wz, start=True, stop=True)

        col = 0
        for s, Wd in enumerate(SUPERS):
            po_t = po_pool.tile([128, 512], DT.float32, tag="po")
            nsl = slice(col, col + Wd)
            per_bank = 512 // Wd
            n_pack = 2 * per_bank     # chunks per [128, 2, 512] tile
            for t in range(0, MCH, n_pack):
                e = pe_pool.tile([128, 2, 512], DT.float32, tag="pe")
                u = upool.tile([128, 2, 512], pv_dt, tag="u")
                for d_ in range(n_pack):
                    tc_ = t + d_
                    ch = slice(tc_ * 128, (tc_ + 1) * 128)
                    nc.tensor.matmul(
                        e[:, d_ // per_bank, (d_ % per_bank) * Wd :][:, :Wd],
                        LA[:, ch],
                        RA[:, nsl],
                        start=True,
                        stop=True,
                    )
                nc.scalar.activation(u, e, AF.Exp)
                for d_ in range(n_pack):
                    tc_ = t + d_
                    nc.tensor.matmul(
                        po_t[:65, :Wd],
                        QA[:, tc_, :],
                        u[:, d_ // per_bank, (d_ % per_bank) * Wd :][:, :Wd],
                        start=(tc_ == 0),
                        stop=(tc_ == MCH - 1),
                    )
            # po_t rows 0..63 = unnormalized out^T, row 64 = S[n]
            ps = opool.tile([65, 512], DT.float32, tag="ps")
            nc.vector.tensor_copy(ps[:, :Wd], po_t[:65, :Wd])
            for j in range(Wd // 128):
                tp = pt_pool.tile([128, 65], DT.float32, tag="tp")
                nc.tensor.transpose(tp, ps[:, j * 128 : (j + 1) * 128], ID[:65, :65])
                rs = spool.tile([128, 1], DT.float32, tag="rs")
                nc.vector.reciprocal(rs, tp[:, 64:65])
                nc.vector.tensor_mul(rs, rs, G)
                of = opool.tile([128, C], DT.float32, tag="of")
                nc.vector.tensor_scalar_mul(of, tp[:, 0:64], rs)
                blk = col // 128 + j
                nc.vector.tensor_add(of, of, XR[:, blk, :])
                nc.sync.dma_start(out=out_r[blk], in_=of)
            col += Wd

    nc.compile()
    return nc


_CACHE = {}


def get_nc():
    key = (QK_MODE, PV_DT)
    if key not in _CACHE:
        _CACHE[key] = _build()
    return _CACHE[key]


def _bf16(a):
    """Round-to-nearest-even float32 -> bfloat16 (as uint16 bit pattern)."""
    u = a.view(np.uint32)
    rounded = ((u + 0x7FFF + ((u >> 16) & 1)) >> 16).astype(np.uint16)
    return rounded


def _bf16_to_f32(b):
    return (b.astype(np.uint32) << 16).view(np.float32)


def make_in_maps(inputs_arr, gamma):
    q_all = np.ascontiguousarray(
        np.asarray(inputs_arr, dtype=np.float32).reshape(B, N, C)
    )
    gv = np.full((128, 1), np.float32(gamma), dtype=np.float32)
    ident = np.eye(128, dtype=np.float32)
    in_maps = []
    for core in range(NCORES):
        b, h = core // 2, core % 2
        qb = q_all[b]                               # (N, C)
        qbT = np.ascontiguousarray(qb.T)            # (C, N)
        sq = np.einsum("nc,nc->n", qb, qb).astype(np.float32)
        r0 = h * R
        m = dict(gvec=gv, ident=ident, x_res=np.ascontiguousarray(qb[r0 : r0 + R]))

        q_aug = np.empty((N, 65), np.float32)
        q_aug[:, :64] = qb
        q_aug[:, 64] = 1.0
        m["q_aug"] = q_aug

        if QK_MODE == "f32r":
            lhs_a = np.empty((65, N), np.float32)
            lhs_a[:64] = qbT
            lhs_a[64] = -1.0
            rhs_a = np.empty((65, R), np.float32)
            rhs_a[:64] = qbT[:, r0 : r0 + R]
            rhs_a[64] = sq[r0 : r0 + R]
            m["lhs_a"], m["rhs_a"] = lhs_a, rhs_a
        else:
            hiT = _bf16(qbT)                        # (64, N) uint16 bf16 bits
            if QK_MODE == "bf16_split":
                loT = _bf16(qbT - _bf16_to_f32(hiT))
                lhs_hl = np.concatenate([hiT, loT], axis=0)       # (128, N)
                rhs_hh = np.concatenate(
                    [hiT[:, r0 : r0 + R], hiT[:, r0 : r0 + R]], axis=0
                )
                lhs_a = np.concatenate(
                    [hiT, _bf16(np.full((1, N), -1.0, np.float32))], axis=0
                )
                rhs_a = np.concatenate(
                    [loT[:, r0 : r0 + R], _bf16(sq[None, r0 : r0 + R])], axis=0
                )
                m["lhs_hl"], m["rhs_hh"] = lhs_hl, rhs_hh
                m["lhs_a"], m["rhs_a"] = lhs_a, rhs_a
            else:  # plain bf16
                lhs_a = np.concatenate(
                    [hiT, _bf16(np.full((1, N), -1.0, np.float32))], axis=0
                )
                rhs_a = np.concatenate(
                    [hiT[:, r0 : r0 + R], _bf16(sq[None, r0 : r0 + R])], axis=0
                )
                m["lhs_a"], m["rhs_a"] = lhs_a, rhs_a
        in_maps.append(m)
    return in_maps


def run_hw(in_maps, **kwargs):
    nc = get_nc()
    return bass_utils.run_bass_kernel_spmd(
        nc, in_maps, core_ids=list(range(NCORES)), **kwargs
    )


def assemble(results):
    out_full = np.empty((B, N, C), np.float32)
    for core in range(NCORES):
        b, h = core // 2, core % 2
        out_full[b, h * R : (h + 1) * R] = results[core]["out"]
    return out_full.reshape(B, D, H, W_, C)


def kernel(**inputs):
    inputs_arr = np.asarray(inputs["inputs"], dtype=np.float32)
    gamma = np.asarray(inputs["gamma"], dtype=np.float32).reshape(-1)[0]
    in_maps = make_in_maps(inputs_arr, gamma)
    try:
        res = run_hw(in_maps)
    except Exception:
        import time

        time.sleep(5)
        res = run_hw(in_maps)
    return assemble(res.results)


# revision 36
# speedup vs baseline: 1.2400x; 1.0027x over previous
"""Channel-attention kernel for Trainium2 (8 NeuronCores).

Reference computation (per batch b):
    q = inputs[b].reshape(N, C)              # N = D*H*W = 4608, C = 64
    E = q @ q.T                              # (N, N)
    A = softmax(E, axis=-1)
    out[b] = gamma * (A @ q) + inputs[b]

Sharding: 8 cores = 4 batches x 2 row-halves of the attention matrix.
Each core computes softmax rows [r0, r0+2304) for one batch. Softmax is
row-wise, so no cross-core communication is needed.

Per-core algorithm (single pass over the attention matrix, transposed
layout, no on-chip transposes of the big matrix):
  * E'^T[m, n] = E[n, m] - sq[n] is computed directly by the QK^T matmul
    with an augmented contraction row: lhsT = [q^T; -1], rhs = [q^T; sq],
    where sq[n] = ||q_n||^2 stabilizes exp (the diagonal is ~the row max,
    and any per-row offset cancels in the softmax normalization).
  * U^T = exp(E'^T) on ScalarE straight out of PSUM (2 chunks per
    ACTIVATE to amortize the ~290-cycle fixed cost).
  * PV matmul with lhsT = [q | 1] computes both the unnormalized output
    (rows 0..63) and the softmax denominator S[n] (row 64) in one PSUM
    accumulation group.
  * Small PE transpose back to [n, c] layout, then out = U/S * gamma + x.
"""

import sys

for _p in ("/opt/trn_rl_repo",):
    if _p not in sys.path:
        sys.path.insert(0, _p)

import numpy as np
from contextlib import ExitStack

import concourse.bacc as bacc
import concourse.tile as tile
from concourse import mybir
from concourse import bass_utils

B, D, H, W_, C = 4, 8, 24, 24, 64
N = D * H * W_            # 4608
NCORES = 8
R = N // 2                # 2304 softmax rows per core
MCH = N // 128            # 36 contraction chunks
SUPERS = [512, 512, 512, 512, 256]   # n-column superblocks (sum = R)
DT = mybir.dt
AF = mybir.ActivationFunctionType

# QK_MODE: "f32r" = one float32r matmul with aug row (fast when PE warm,
#          ~fp32 accuracy); "bf16_split" = q split into bf16 hi+lo, two
#          bf16 matmuls (fp32-like accuracy); "bf16" = one bf16 matmul
#          (fastest, ~1% error).
QK_MODE = "f32r"
PV_DT = DT.float32r       # dtype of U tiles + q_aug for the PV matmul


def _build(qk_mode=None, pv_dt=None):
    qk_mode = qk_mode or QK_MODE
    pv_dt = pv_dt or PV_DT
    if qk_mode == "bf16_split":
        raise NotImplementedError("bf16_split dropped in asymmetric-psum layout")
    bf = DT.bfloat16
    nc = bacc.Bacc("TRN2", target_bir_lowering=False, debug=False)

    if qk_mode == "f32r":
        lhs_a = nc.dram_tensor("lhs_a", (65, N), DT.float32, kind="ExternalInput").ap()
        rhs_a = nc.dram_tensor("rhs_a", (65, R), DT.float32, kind="ExternalInput").ap()
    elif qk_mode == "bf16_split":
        lhs_hl = nc.dram_tensor("lhs_hl", (128, N), bf, kind="ExternalInput").ap()
        rhs_hh = nc.dram_tensor("rhs_hh", (128, R), bf, kind="ExternalInput").ap()
        lhs_a = nc.dram_tensor("lhs_a", (65, N), bf, kind="ExternalInput").ap()
        rhs_a = nc.dram_tensor("rhs_a", (65, R), bf, kind="ExternalInput").ap()
    elif qk_mode == "bf16":
        lhs_a = nc.dram_tensor("lhs_a", (65, N), bf, kind="ExternalInput").ap()
        rhs_a = nc.dram_tensor("rhs_a", (65, R), bf, kind="ExternalInput").ap()
    else:
        raise ValueError(qk_mode)

    q_aug = nc.dram_tensor("q_aug", (N, 65), DT.float32, kind="ExternalInput").ap()
    x_res = nc.dram_tensor("x_res", (R, C), DT.float32, kind="ExternalInput").ap()
    gvec = nc.dram_tensor("gvec", (128, 1), DT.float32, kind="ExternalInput").ap()
    ident = nc.dram_tensor("ident", (128, 128), DT.float32, kind="ExternalInput").ap()
    out = nc.dram_tensor("out", (R, C), DT.float32, kind="ExternalOutput").ap()

    qk_dt = DT.float32r if qk_mode == "f32r" else bf

    with ExitStack() as ctx:
        tc = ctx.enter_context(tile.TileContext(nc))
        singles = ctx.enter_context(tc.tile_pool(name="singles", bufs=1))
        upool = ctx.enter_context(tc.tile_pool(name="u", bufs=6))
        opool = ctx.enter_context(tc.tile_pool(name="o", bufs=4))
        spool = ctx.enter_context(tc.tile_pool(name="s", bufs=4))
        # PSUM budget (8 banks): pe 3x2 + po 1 + pt 1 — the 3-deep pe pool
        # lets the PE run up to 3 chunk-pairs ahead of ScalarE, which keeps
        # both engines packed. Measured better than pe 2x2/po 2/pt 2 (122us
        # -> 109us) and than a single-buffered 4-bank/2048-wide-EXP layout
        # (131us: the wider EXP saves ~7us of ACT work but loses the
        # lookahead depth, leaving ScalarE only ~70% occupied).
        pe_pool = ctx.enter_context(tc.tile_pool(name="pe", bufs=3, space="PSUM"))
        po_pool = ctx.enter_context(tc.tile_pool(name="po", bufs=1, space="PSUM"))
        pt_pool = ctx.enter_context(tc.tile_pool(name="pt", bufs=1, space="PSUM"))

        # Stage input DMAs in first-use order across two rings (sync HWDGE
        # for the QK operands, gpsimd SWDGE for the PV/epilogue operands)
        # so the main loop can start ~4us in instead of waiting ~15us for
        # everything.
        la_src = lhs_a.bitcast(qk_dt) if qk_mode == "f32r" else lhs_a
        ra_src = rhs_a.bitcast(qk_dt) if qk_mode == "f32r" else rhs_a
        LA = singles.tile([65, N], qk_dt)
        RA = singles.tile([65, R], qk_dt)
        nc.sync.dma_start(out=RA[:, :512], in_=ra_src[:, :512])
        for a in range(0, MCH, 9):
            sl = slice(a * 128, (a + 9) * 128)
            nc.sync.dma_start(out=LA[:, sl], in_=la_src[:, sl])
        nc.sync.dma_start(out=RA[:, 512:], in_=ra_src[:, 512:])
        if qk_mode == "bf16_split":
            LHL = singles.tile([128, N], bf)
            nc.sync.dma_start(out=LHL, in_=lhs_hl)
            RHH = singles.tile([128, R], bf)
            nc.sync.dma_start(out=RHH, in_=rhs_hh)

        qa_src = (
            q_aug.bitcast(DT.float32r) if pv_dt == DT.float32r else q_aug
        ).rearrange("(t p) c -> p t c", p=128)
        QA = singles.tile([128, MCH, 65], pv_dt)
        for a in range(0, MCH, 6):
            nc.gpsimd.dma_start(out=QA[:, a : a + 6, :], in_=qa_src[:, a : a + 6, :])
        ID = singles.tile([128, 128], DT.float32)
        nc.gpsimd.dma_start(out=ID, in_=ident)
        G = singles.tile([128, 1], DT.float32)
        nc.gpsimd.dma_start(out=G, in_=gvec)
        XR = singles.tile([128, R // 128, C], DT.float32)
        nc.gpsimd.dma_start(out=XR, in_=x_res.rearrange("(t p) c -> p t c", p=128))

        out_r = out.rearrange("(t p) c -> t p c", p=128)

        # HAM warmup: fp32-mode matmuls don't un-throttle the PE clock
        # (observed: all-fp32r kernel runs at 1.2 GHz forever). Issue ~5us
        # of bf16 dummy matmuls first — they overlap the input DMAs, warm
        # the PE to 2.4 GHz, and the fp32r stream then retains warmth.
        wz = singles.tile([128, 512], DT.bfloat16)
        nc.vector.memset(wz, 0.0)
        # Pre-load the Exp spline table off the critical path (overlaps the
        # input DMAs) so group 0's first EXP doesn't stall ~2.7us.
        sc0 = spool.tile([128, 1], DT.float32, tag="sc0")
        nc.vector.memset(sc0, 0.0)
        tl0 = spool.tile([128, 1], DT.float32, tag="tl0")
        nc.scalar.activation(tl0, sc0, AF.Exp)
        for w_ in range(4):
            ew = pe_pool.tile([128, 2, 512], DT.float32, tag="pe")
            for k_ in range(2):
                nc.tensor.matmul(ew[:, k_, :], wz[:, :128], wz, start=True, stop=True)

        col = 0
        for s, Wd in enumerate(SUPERS):
            po_t = po_pool.tile([128, 512], DT.float32, tag="po")
            nsl = slice(col, col + Wd)
            per_bank = 512 // Wd
            n_pack = 2 * per_bank     # chunks per [128, 2, 512] tile
            for t in range(0, MCH, n_pack):
                e = pe_pool.tile([128, 2, 512], DT.float32, tag="pe")
                u = upool.tile([128, 2, 512], pv_dt, tag="u")
                for d_ in range(n_pack):
                    tc_ = t + d_
                    ch = slice(tc_ * 128, (tc_ + 1) * 128)
                    nc.tensor.matmul(
                        e[:, d_ // per_bank, (d_ % per_bank) * Wd :][:, :Wd],
                        LA[:, ch],
                        RA[:, nsl],
                        start=True,
                        stop=True,
                    )
                nc.scalar.activation(u, e, AF.Exp)
                for d_ in range(n_pack):
                    tc_ = t + d_
                    nc.tensor.matmul(
                        po_t[:65, :Wd],
                        QA[:, tc_, :],
                        u[:, d_ // per_bank, (d_ % per_bank) * Wd :][:, :Wd],
                        start=(tc_ == 0),
                        stop=(tc_ == MCH - 1),
                    )
            # po_t rows 0..63 = unnormalized out^T, row 64 = S[n]
            ps = opool.tile([65, 512], DT.float32, tag="ps")
            nc.vector.tensor_copy(ps[:, :Wd], po_t[:65, :Wd])
            for j in range(Wd // 128):
                tp = pt_pool.tile([128, 65], DT.float32, tag="tp")
                nc.tensor.transpose(tp, ps[:, j * 128 : (j + 1) * 128], ID[:65, :65])
                rs = spool.tile([128, 1], DT.float32, tag="rs")
                nc.vector.reciprocal(rs, tp[:, 64:65])
                nc.vector.tensor_mul(rs, rs, G)
                of = opool.tile([128, C], DT.float32, tag="of")
                nc.vector.tensor_scalar_mul(of, tp[:, 0:64], rs)
                blk = col // 128 + j
                nc.vector.tensor_add(of, of, XR[:, blk, :])
                nc.sync.dma_start(out=out_r[blk], in_=of)
            col += Wd

    nc.compile()
    return nc


_CACHE = {}


def get_nc():
    key = (QK_MODE, PV_DT)
    if key not in _CACHE:
        _CACHE[key] = _build()
    return _CACHE[key]


def _bf16(a):
    """Round-to-nearest-even float32 -> bfloat16 (as uint16 bit pattern)."""
    u = a.view(np.uint32)
    rounded = ((u + 0x7FFF + ((u >> 16) & 1)) >> 16).astype(np.uint16)
    return rounded


def _bf16_to_f32(b):
    return (b.astype(np.uint32) << 16).view(np.float32)


def make_in_maps(inputs_arr, gamma):
    q_all = np.ascontiguousarray(
        np.asarray(inputs_arr, dtype=np.float32).reshape(B, N, C)
    )
    gv = np.full((128, 1), np.float32(gamma), dtype=np.float32)
    ident = np.eye(128, dtype=np.float32)
    in_maps = []
    for core in range(NCORES):
        b, h = core // 2, core % 2
        qb = q_all[b]                               # (N, C)
        qbT = np.ascontiguousarray(qb.T)            # (C, N)
        sq = np.einsum("nc,nc->n", qb, qb).astype(np.float32)
        r0 = h * R
        m = dict(gvec=gv, ident=ident, x_res=np.ascontiguousarray(qb[r0 : r0 + R]))

        q_aug = np.empty((N, 65), np.float32)
        q_aug[:, :64] = qb
        q_aug[:, 64] = 1.0
        m["q_aug"] = q_aug

        if QK_MODE == "f32r":
            lhs_a = np.empty((65, N), np.float32)
            lhs_a[:64] = qbT
            lhs_a[64] = -1.0
            rhs_a = np.empty((65, R), np.float32)
            rhs_a[:64] = qbT[:, r0 : r0 + R]
            rhs_a[64] = sq[r0 : r0 + R]
            m["lhs_a"], m["rhs_a"] = lhs_a, rhs_a
        else:
            hiT = _bf16(qbT)                        # (64, N) uint16 bf16 bits
            if QK_MODE == "bf16_split":
                loT = _bf16(qbT - _bf16_to_f32(hiT))
                lhs_hl = np.concatenate([hiT, loT], axis=0)       # (128, N)
                rhs_hh = np.concatenate(
                    [hiT[:, r0 : r0 + R], hiT[:, r0 : r0 + R]], axis=0
                )
                lhs_a = np.concatenate(
                    [hiT, _bf16(np.full((1, N), -1.0, np.float32))], axis=0
                )
                rhs_a = np.concatenate(
                    [loT[:, r0 : r0 + R], _bf16(sq[None, r0 : r0 + R])], axis=0
                )
                m["lhs_hl"], m["rhs_hh"] = lhs_hl, rhs_hh
                m["lhs_a"], m["rhs_a"] = lhs_a, rhs_a
            else:  # plain bf16
                lhs_a = np.concatenate(
                    [hiT, _bf16(np.full((1, N), -1.0, np.float32))], axis=0
                )
                rhs_a = np.concatenate(
                    [hiT[:, r0 : r0 + R], _bf16(sq[None, r0 : r0 + R])], axis=0
                )
                m["lhs_a"], m["rhs_a"] = lhs_a, rhs_a
        in_maps.append(m)
    return in_maps


def run_hw(in_maps, **kwargs):
    nc = get_nc()
    return bass_utils.run_bass_kernel_spmd(
        nc, in_maps, core_ids=list(range(NCORES)), **kwargs
    )


def assemble(results):
    out_full = np.empty((B, N, C), np.float32)
    for core in range(NCORES):
        b, h = core // 2, core % 2
        out_full[b, h * R : (h + 1) * R] = results[core]["out"]
    return out_full.reshape(B, D, H, W_, C)


def kernel(**inputs):
    inputs_arr = np.asarray(inputs["inputs"], dtype=np.float32)
    gamma = np.asarray(inputs["gamma"], dtype=np.float32).reshape(-1)[0]
    in_maps = make_in_maps(inputs_arr, gamma)
    try:
        res = run_hw(in_maps)
    except Exception:
        import time

        time.sleep(5)
        res = run_hw(in_maps)
    return assemble(res.results)


# revision 37
# speedup vs baseline: 1.2428x; 1.0022x over previous
"""Channel-attention kernel for Trainium2 (8 NeuronCores).

Reference computation (per batch b):
    q = inputs[b].reshape(N, C)              # N = D*H*W = 4608, C = 64
    E = q @ q.T                              # (N, N)
    A = softmax(E, axis=-1)
    out[b] = gamma * (A @ q) + inputs[b]

Sharding: 8 cores = 4 batches x 2 row-halves of the attention matrix.
Each core computes softmax rows [r0, r0+2304) for one batch. Softmax is
row-wise, so no cross-core communication is needed.

Per-core algorithm (single pass over the attention matrix, transposed
layout, no on-chip transposes of the big matrix):
  * E'^T[m, n] = E[n, m] - sq[n] is computed directly by the QK^T matmul
    with an augmented contraction row: lhsT = [q^T; -1], rhs = [q^T; sq],
    where sq[n] = ||q_n||^2 stabilizes exp (the diagonal is ~the row max,
    and any per-row offset cancels in the softmax normalization).
  * U^T = exp(E'^T) on ScalarE straight out of PSUM (2 chunks per
    ACTIVATE to amortize the ~290-cycle fixed cost).
  * PV matmul with lhsT = [q | 1] computes both the unnormalized output
    (rows 0..63) and the softmax denominator S[n] (row 64) in one PSUM
    accumulation group.
  * Small PE transpose back to [n, c] layout, then out = U/S * gamma + x.
"""

import sys

for _p in ("/opt/trn_rl_repo",):
    if _p not in sys.path:
        sys.path.insert(0, _p)

import numpy as np
from contextlib import ExitStack

import concourse.bacc as bacc
import concourse.tile as tile
from concourse import mybir
from concourse import bass_utils

B, D, H, W_, C = 4, 8, 24, 24, 64
N = D * H * W_            # 4608
NCORES = 8
R = N // 2                # 2304 softmax rows per core
MCH = N // 128            # 36 contraction chunks
SUPERS = [512, 512, 512, 512, 256]   # n-column superblocks (sum = R)
DT = mybir.dt
AF = mybir.ActivationFunctionType

# QK_MODE: "f32r" = one float32r matmul with aug row (fast when PE warm,
#          ~fp32 accuracy); "bf16_split" = q split into bf16 hi+lo, two
#          bf16 matmuls (fp32-like accuracy); "bf16" = one bf16 matmul
#          (fastest, ~1% error).
QK_MODE = "f32r"
PV_DT = DT.float32r       # dtype of U tiles + q_aug for the PV matmul


def _build(qk_mode=None, pv_dt=None):
    qk_mode = qk_mode or QK_MODE
    pv_dt = pv_dt or PV_DT
    if qk_mode == "bf16_split":
        raise NotImplementedError("bf16_split dropped in asymmetric-psum layout")
    bf = DT.bfloat16
    nc = bacc.Bacc("TRN2", target_bir_lowering=False, debug=False)

    if qk_mode == "f32r":
        lhs_a = nc.dram_tensor("lhs_a", (65, N), DT.float32, kind="ExternalInput").ap()
        rhs_a = nc.dram_tensor("rhs_a", (65, R), DT.float32, kind="ExternalInput").ap()
    elif qk_mode == "bf16_split":
        lhs_hl = nc.dram_tensor("lhs_hl", (128, N), bf, kind="ExternalInput").ap()
        rhs_hh = nc.dram_tensor("rhs_hh", (128, R), bf, kind="ExternalInput").ap()
        lhs_a = nc.dram_tensor("lhs_a", (65, N), bf, kind="ExternalInput").ap()
        rhs_a = nc.dram_tensor("rhs_a", (65, R), bf, kind="ExternalInput").ap()
    elif qk_mode == "bf16":
        lhs_a = nc.dram_tensor("lhs_a", (65, N), bf, kind="ExternalInput").ap()
        rhs_a = nc.dram_tensor("rhs_a", (65, R), bf, kind="ExternalInput").ap()
    else:
        raise ValueError(qk_mode)

    q_aug = nc.dram_tensor("q_aug", (N, 65), DT.float32, kind="ExternalInput").ap()
    x_res = nc.dram_tensor("x_res", (R, C), DT.float32, kind="ExternalInput").ap()
    gvec = nc.dram_tensor("gvec", (128, 1), DT.float32, kind="ExternalInput").ap()
    ident = nc.dram_tensor("ident", (128, 128), DT.float32, kind="ExternalInput").ap()
    out = nc.dram_tensor("out", (R, C), DT.float32, kind="ExternalOutput").ap()

    qk_dt = DT.float32r if qk_mode == "f32r" else bf

    with ExitStack() as ctx:
        tc = ctx.enter_context(tile.TileContext(nc))
        singles = ctx.enter_context(tc.tile_pool(name="singles", bufs=1))
        upool = ctx.enter_context(tc.tile_pool(name="u", bufs=6))
        opool = ctx.enter_context(tc.tile_pool(name="o", bufs=4))
        spool = ctx.enter_context(tc.tile_pool(name="s", bufs=4))
        # PSUM budget (8 banks): pe 3x2 + po 1 + pt 1 — the 3-deep pe pool
        # lets the PE run up to 3 chunk-pairs ahead of ScalarE, which keeps
        # both engines packed. Measured better than pe 2x2/po 2/pt 2 (122us
        # -> 109us) and than a single-buffered 4-bank/2048-wide-EXP layout
        # (131us: the wider EXP saves ~7us of ACT work but loses the
        # lookahead depth, leaving ScalarE only ~70% occupied).
        pe_pool = ctx.enter_context(tc.tile_pool(name="pe", bufs=3, space="PSUM"))
        po_pool = ctx.enter_context(tc.tile_pool(name="po", bufs=1, space="PSUM"))
        pt_pool = ctx.enter_context(tc.tile_pool(name="pt", bufs=1, space="PSUM"))

        # Stage input DMAs in first-use order across two rings (sync HWDGE
        # for the QK operands, gpsimd SWDGE for the PV/epilogue operands)
        # so the main loop can start ~4us in instead of waiting ~15us for
        # everything.
        la_src = lhs_a.bitcast(qk_dt) if qk_mode == "f32r" else lhs_a
        ra_src = rhs_a.bitcast(qk_dt) if qk_mode == "f32r" else rhs_a
        LA = singles.tile([65, N], qk_dt)
        RA = singles.tile([65, R], qk_dt)
        nc.sync.dma_start(out=RA[:, :512], in_=ra_src[:, :512])
        for a in range(0, MCH, 9):
            sl = slice(a * 128, (a + 9) * 128)
            nc.sync.dma_start(out=LA[:, sl], in_=la_src[:, sl])
        nc.sync.dma_start(out=RA[:, 512:], in_=ra_src[:, 512:])
        if qk_mode == "bf16_split":
            LHL = singles.tile([128, N], bf)
            nc.sync.dma_start(out=LHL, in_=lhs_hl)
            RHH = singles.tile([128, R], bf)
            nc.sync.dma_start(out=RHH, in_=rhs_hh)

        qa_src = (
            q_aug.bitcast(DT.float32r) if pv_dt == DT.float32r else q_aug
        ).rearrange("(t p) c -> p t c", p=128)
        QA = singles.tile([128, MCH, 65], pv_dt)
        for a in range(0, MCH, 6):
            nc.gpsimd.dma_start(out=QA[:, a : a + 6, :], in_=qa_src[:, a : a + 6, :])
        ID = singles.tile([128, 128], DT.float32)
        nc.gpsimd.dma_start(out=ID, in_=ident)
        G = singles.tile([128, 1], DT.float32)
        nc.gpsimd.dma_start(out=G, in_=gvec)
        XR = singles.tile([128, R // 128, C], DT.float32)
        nc.gpsimd.dma_start(out=XR, in_=x_res.rearrange("(t p) c -> p t c", p=128))

        out_r = out.rearrange("(t p) c -> t p c", p=128)

        # HAM warmup: fp32-mode matmuls don't un-throttle the PE clock
        # (observed: all-fp32r kernel runs at 1.2 GHz forever). Issue ~5us
        # of bf16 dummy matmuls first — they overlap the input DMAs, warm
        # the PE to 2.4 GHz, and the fp32r stream then retains warmth.
        wz = singles.tile([128, 512], DT.bfloat16)
        nc.vector.memset(wz, 0.0)
        # Pre-load the Exp spline table off the critical path (overlaps the
        # input DMAs) so group 0's first EXP doesn't stall ~2.7us.
        sc0 = spool.tile([128, 1], DT.float32, tag="sc0")
        nc.vector.memset(sc0, 0.0)
        tl0 = spool.tile([128, 1], DT.float32, tag="tl0")
        nc.scalar.activation(tl0, sc0, AF.Exp)
        for w_ in range(4):
            ew = pe_pool.tile([128, 2, 512], DT.float32, tag="pe")
            for k_ in range(2):
                nc.tensor.matmul(ew[:, k_, :], wz[:, :128], wz, start=True, stop=True)

        def epilogue(po_t, col_, Wd_):
            # po_t rows 0..63 = unnormalized out^T, row 64 = S[n]
            ps = opool.tile([65, 512], DT.float32, tag="ps", name="ps")
            nc.vector.tensor_copy(ps[:, :Wd_], po_t[:65, :Wd_])
            for j in range(Wd_ // 128):
                tp = pt_pool.tile([128, 65], DT.float32, tag="tp", name="tp")
                nc.tensor.transpose(tp, ps[:, j * 128 : (j + 1) * 128], ID[:65, :65])
                rs = spool.tile([128, 1], DT.float32, tag="rs", name="rs")
                nc.vector.reciprocal(rs, tp[:, 64:65])
                nc.vector.tensor_mul(rs, rs, G)
                of = opool.tile([128, C], DT.float32, tag="of", name="of")
                nc.vector.tensor_scalar_mul(of, tp[:, 0:64], rs)
                blk = col_ // 128 + j
                nc.vector.tensor_add(of, of, XR[:, blk, :])
                nc.sync.dma_start(out=out_r[blk], in_=of)

        # Each super's epilogue (copy + transposes + normalize + DMA) is
        # emitted after the NEXT super's first group, so the PE-side
        # transposes fall into natural slack instead of delaying the next
        # super's QK/EXP stream at the boundary.
        pending = None
        col = 0
        for s, Wd in enumerate(SUPERS):
            po_t = po_pool.tile([128, 512], DT.float32, tag="po")
            nsl = slice(col, col + Wd)
            per_bank = 512 // Wd
            n_pack = 2 * per_bank     # chunks per [128, 2, 512] tile
            for t in range(0, MCH, n_pack):
                e = pe_pool.tile([128, 2, 512], DT.float32, tag="pe")
                u = upool.tile([128, 2, 512], pv_dt, tag="u")
                for d_ in range(n_pack):
                    tc_ = t + d_
                    ch = slice(tc_ * 128, (tc_ + 1) * 128)
                    nc.tensor.matmul(
                        e[:, d_ // per_bank, (d_ % per_bank) * Wd :][:, :Wd],
                        LA[:, ch],
                        RA[:, nsl],
                        start=True,
                        stop=True,
                    )
                nc.scalar.activation(u, e, AF.Exp)
                for d_ in range(n_pack):
                    tc_ = t + d_
                    nc.tensor.matmul(
                        po_t[:65, :Wd],
                        QA[:, tc_, :],
                        u[:, d_ // per_bank, (d_ % per_bank) * Wd :][:, :Wd],
                        start=(tc_ == 0),
                        stop=(tc_ == MCH - 1),
                    )
                if t == 0 and pending is not None:
                    epilogue(*pending)
                    pending = None
            pending = (po_t, col, Wd)
            col += Wd
        epilogue(*pending)

    nc.compile()
    return nc


_CACHE = {}


def get_nc():
    key = (QK_MODE, PV_DT)
    if key not in _CACHE:
        _CACHE[key] = _build()
    return _CACHE[key]


def _bf16(a):
    """Round-to-nearest-even float32 -> bfloat16 (as uint16 bit pattern)."""
    u = a.view(np.uint32)
    rounded = ((u + 0x7FFF + ((u >> 16) & 1)) >> 16).astype(np.uint16)
    return rounded


def _bf16_to_f32(b):
    return (b.astype(np.uint32) << 16).view(np.float32)


def make_in_maps(inputs_arr, gamma):
    q_all = np.ascontiguousarray(
        np.asarray(inputs_arr, dtype=np.float32).reshape(B, N, C)
    )
    gv = np.full((128, 1), np.float32(gamma), dtype=np.float32)
    ident = np.eye(128, dtype=np.float32)
    in_maps = []
    for core in range(NCORES):
        b, h = core // 2, core % 2
        qb = q_all[b]                               # (N, C)
        qbT = np.ascontiguousarray(qb.T)            # (C, N)
        sq = np.einsum("nc,nc->n", qb, qb).astype(np.float32)
        r0 = h * R
        m = dict(gvec=gv, ident=ident, x_res=np.ascontiguousarray(qb[r0 : r0 + R]))

        q_aug = np.empty((N, 65), np.float32)
        q_aug[:, :64] = qb
        q_aug[:, 64] = 1.0
        m["q_aug"] = q_aug

        if QK_MODE == "f32r":
            lhs_a = np.empty((65, N), np.float32)
            lhs_a[:64] = qbT
            lhs_a[64] = -1.0
            rhs_a = np.empty((65, R), np.float32)
            rhs_a[:64] = qbT[:, r0 : r0 + R]
            rhs_a[64] = sq[r0 : r0 + R]
            m["lhs_a"], m["rhs_a"] = lhs_a, rhs_a
        else:
            hiT = _bf16(qbT)                        # (64, N) uint16 bf16 bits
            if QK_MODE == "bf16_split":
                loT = _bf16(qbT - _bf16_to_f32(hiT))
                lhs_hl = np.concatenate([hiT, loT], axis=0)       # (128, N)
                rhs_hh = np.concatenate(
                    [hiT[:, r0 : r0 + R], hiT[:, r0 : r0 + R]], axis=0
                )
                lhs_a = np.concatenate(
                    [hiT, _bf16(np.full((1, N), -1.0, np.float32))], axis=0
                )
                rhs_a = np.concatenate(
                    [loT[:, r0 : r0 + R], _bf16(sq[None, r0 : r0 + R])], axis=0
                )
                m["lhs_hl"], m["rhs_hh"] = lhs_hl, rhs_hh
                m["lhs_a"], m["rhs_a"] = lhs_a, rhs_a
            else:  # plain bf16
                lhs_a = np.concatenate(
                    [hiT, _bf16(np.full((1, N), -1.0, np.float32))], axis=0
                )
                rhs_a = np.concatenate(
                    [hiT[:, r0 : r0 + R], _bf16(sq[None, r0 : r0 + R])], axis=0
                )
                m["lhs_a"], m["rhs_a"] = lhs_a, rhs_a
        in_maps.append(m)
    return in_maps


def run_hw(in_maps, **kwargs):
    nc = get_nc()
    return bass_utils.run_bass_kernel_spmd(
        nc, in_maps, core_ids=list(range(NCORES)), **kwargs
    )


def assemble(results):
    out_full = np.empty((B, N, C), np.float32)
    for core in range(NCORES):
        b, h = core // 2, core % 2
        out_full[b, h * R : (h + 1) * R] = results[core]["out"]
    return out_full.reshape(B, D, H, W_, C)


def kernel(**inputs):
    inputs_arr = np.asarray(inputs["inputs"], dtype=np.float32)
    gamma = np.asarray(inputs["gamma"], dtype=np.float32).reshape(-1)[0]
    in_maps = make_in_maps(inputs_arr, gamma)
    try:
        res = run_hw(in_maps)
    except Exception:
        import time

        time.sleep(5)
        res = run_hw(in_maps)
    return assemble(res.results)
